# revision 1
# baseline (speedup 1.0000x reference)
"""Contextual-attention Trainium2 kernel (Bass/Tile), data-parallel over batch.

Math (per sequence b):
    Q = evo @ q_w.T + q_b                                  (L, 96)
    K = cat(evo, conv3(evo), conv5(evo)) @ k_w.T + k_b     (L, 96)
    V = plm @ v_w.T + v_b                                  (L, 96)
    P = softmax(Q K^T / sqrt(96), key-masked by seqlen)
    out = P @ V + V

Device-side reformulation (per core = one sequence):
  * The two convs + concat + K-projection fold into 5 shifted matmuls:
        K[l] = sum_{t=-2..2} evo[l+t] @ taps[t]  + bk      (host-folded weights)
  * Everything is computed transposed ([feature, L] layout) so the only
    contraction layouts needed are natural ones:
        QT = wqT.T @ evoT, KT = taps.T @ evoT(shifted), VT = wvT.T @ plmT
        ST[lk, lq] = KT_slice.T @ QT  -> exp via ACT with per-partition mask bias
        OT[0:96]   = sum_lk V1[lk].T @ ET[lk]   (V1 = [V | ones] natural layout,
        OT[96]     = softmax denominator         via on-chip PE transpose of VT)
  * All matmul operands are fp16 (PE streams 2B/cycle: fp32 is half rate), all
    accumulation is f32 in PSUM; exp runs in f32 on ScalarE. fp16 (not bf16)
    because every tensor here is O(1)-ranged and fp16 carries 3 more mantissa
    bits.
  * Key tiles entirely beyond max(seqlen) are skipped at build time; the
    per-core mask bias (0 / -1e6) zeroes partially-valid tiles exactly
    (exp(-1e6 + s) underflows to 0.0f, matching the reference's where()+softmax).
  * Final divide by denominator, +V residual, and the (96, L) -> (L, 96)
    transpose happen on host (tiny O(L*96) work).
"""

import os
import numpy as np

import concourse.bacc as bacc
import concourse.bass as bass
import concourse.tile as tile
from concourse import mybir
from concourse._compat import get_trn_type
from concourse.bass_utils import run_bass_kernel_spmd

B, L = 8, 2048
Q_IN, V_IN, QK, VD = 512, 1024, 96, 96
P = 128
NORM = float(1.0 / np.sqrt(QK))
F32 = mybir.dt.float32
F16 = mybir.dt.float16

LAST_EXEC_TIME_NS = None
LAST_RESULTS = None

_program_cache = {}


def _fold_k_weights(k_w, k_b, cn3_w, cn3_b, cn5_w, cn5_b):
    """K[l] = sum_{t in -2..2} evo[l+t] @ taps[t+2] + bk  (zero-padded shifts)."""
    A_evo = k_w[:, :Q_IN]
    A3 = k_w[:, Q_IN : Q_IN + VD]
    A5 = k_w[:, Q_IN + VD :]
    taps = np.zeros((5, Q_IN, QK), np.float32)
    for j in range(3):  # conv3 tap j acts at offset t = j-1
        taps[j - 1 + 2] += np.einsum("oc,cd->do", A3, cn3_w[:, :, j]).astype(np.float32)
    for j in range(5):  # conv5 tap j acts at offset t = j-2
        taps[j - 2 + 2] += np.einsum("oc,cd->do", A5, cn5_w[:, :, j]).astype(np.float32)
    taps[2] += A_evo.T
    bk = (k_b + A3 @ cn3_b + A5 @ cn5_b).astype(np.float32)
    return taps, bk


def _chunks(total, step=512):
    out = []
    o = 0
    while o < total:
        out.append((o, min(step, total - o)))
        o += step
    return out


def _build_program(nkt):
    """One SPMD program; all cores run NKT key tiles, masks differ per core."""
    lkw = nkt * P
    nc = bacc.Bacc(get_trn_type() or "TRN2", target_bir_lowering=False, debug=False)
    # weight/constant params (tiny, loaded first)
    wq = nc.declare_dram_parameter("wq", [P, 4 * QK], F16, isOutput=False)
    wk = nc.declare_dram_parameter("wk", [P, 20 * QK], F16, isOutput=False)
    wv = nc.declare_dram_parameter("wv", [P, 8 * QK], F16, isOutput=False)
    bqkv = nc.declare_dram_parameter("bqkv", [QK, 3], F32, isOutput=False)
    maskd = nc.declare_dram_parameter("mask", [P, nkt], F32, isOutput=False)
    identd = nc.declare_dram_parameter("ident", [P, P], F16, isOutput=False)
    # activations
    evoT = nc.declare_dram_parameter("evoT", [Q_IN, L + 4], F16, isOutput=False)
    plmT = nc.declare_dram_parameter("plmT", [V_IN, L], F16, isOutput=False)
    # outputs
    ot_out = nc.declare_dram_parameter("ot", [QK + 1, L], F32, isOutput=True)
    vt_out = nc.declare_dram_parameter("vt", [QK, L], F16, isOutput=True)

    add = mybir.AluOpType.add

    with tile.TileContext(nc) as tc:
        with tc.tile_pool(name="sing", bufs=1) as sing:
            # ---- weights + evo first (they gate the first matmuls); finer
            # partition splits engage more DMA engines in parallel ----
            wq_sb = sing.tile([P, 4, QK], F16, tag="wq")
            nc.sync.dma_start(out=wq_sb, in_=wq[:, :].rearrange("p (n o) -> p n o", o=QK))
            evo_sb = []
            for i in range(4):
                t = sing.tile([P, L + 4], F16, tag=f"evo{i}")
                for h in range(2):
                    nc.sync.dma_start(
                        out=t[h * 64 : (h + 1) * 64, :],
                        in_=evoT[i * P + h * 64 : i * P + (h + 1) * 64, :],
                    )
                evo_sb.append(t)
            wk_sb = sing.tile([P, 20, QK], F16, tag="wk")
            nc.sync.dma_start(out=wk_sb, in_=wk[:, :].rearrange("p (n o) -> p n o", o=QK))
            wv_sb = sing.tile([P, 8, QK], F16, tag="wv")
            nc.sync.dma_start(out=wv_sb, in_=wv[:, :].rearrange("p (n o) -> p n o", o=QK))
            b_sb = sing.tile([QK, 3], F32, tag="bqkv")
            nc.sync.dma_start(out=b_sb, in_=bqkv[:, :])
            mask_sb = sing.tile([P, nkt], F32, tag="mask")
            nc.sync.dma_start(out=mask_sb, in_=maskd[:, :])
            ident_sb = sing.tile([P, P], F16, tag="ident")
            nc.sync.dma_start(out=ident_sb, in_=identd[:, :])
            plm_sb = []
            for i in range(8):
                t = sing.tile([P, L], F16, tag=f"plm{i}")
                for h in range(2):
                    nc.sync.dma_start(
                        out=t[h * 64 : (h + 1) * 64, :],
                        in_=plmT[i * P + h * 64 : i * P + (h + 1) * 64, :],
                    )
                plm_sb.append(t)

            qt_sb = sing.tile([QK, L], F16, tag="qt")
            kt_sb = sing.tile([QK, lkw], F16, tag="kt")
            vt_sb = sing.tile([QK, L], F32, tag="vt")
            vt16_sb = sing.tile([QK, L], F16, tag="vt16")
            v1_sb = sing.tile([P, nkt, QK + 1], F16, tag="v1")
            ot_sb = sing.tile([QK + 1, L], F32, tag="ot")

            # ---- projections ----
            with (
                tc.tile_pool(name="proj_psum", bufs=3, space="PSUM") as proj_psum,
                tc.tile_pool(name="v1_psum", bufs=2, space="PSUM") as v1_psum,
            ):
                # QT = wq.T @ evoT  (+qb)
                for base, width in _chunks(L, 1024):
                    pt = proj_psum.tile([QK, 1024], F32, tag="proj")
                    for dt in range(4):
                        for o2, w2 in _chunks(width, 512):
                            nc.tensor.matmul(
                                pt[:, o2 : o2 + w2],
                                lhsT=wq_sb[:, dt, :],
                                rhs=evo_sb[dt][:, 2 + base + o2 : 2 + base + o2 + w2],
                                start=(dt == 0),
                                stop=(dt == 3),
                            )
                    nc.vector.tensor_scalar(
                        out=qt_sb[:, base : base + width],
                        in0=pt[:, :width],
                        scalar1=b_sb[:, 0:1],
                        scalar2=None,
                        op0=add,
                    )
                # KT = sum_t taps[t].T @ evoT(shift t-2)  (+kb), first lkw cols only
                for base, width in _chunks(lkw, 1024):
                    pt = proj_psum.tile([QK, 1024], F32, tag="proj")
                    n = 0
                    for t in range(5):
                        for dt in range(4):
                            for o2, w2 in _chunks(width, 512):
                                nc.tensor.matmul(
                                    pt[:, o2 : o2 + w2],
                                    lhsT=wk_sb[:, t * 4 + dt, :],
                                    rhs=evo_sb[dt][:, t + base + o2 : t + base + o2 + w2],
                                    start=(n == 0),
                                    stop=(n == 19),
                                )
                            n += 1
                    nc.vector.tensor_scalar(
                        out=kt_sb[:, base : base + width],
                        in0=pt[:, :width],
                        scalar1=b_sb[:, 1:2],
                        scalar2=None,
                        op0=add,
                    )
                # VT = wv.T @ plmT (+vb), full L (residual needs all of V)
                for base, width in _chunks(L, 1024):
                    pt = proj_psum.tile([QK, 1024], F32, tag="proj")
                    for dt in range(8):
                        for o2, w2 in _chunks(width, 512):
                            nc.tensor.matmul(
                                pt[:, o2 : o2 + w2],
                                lhsT=wv_sb[:, dt, :],
                                rhs=plm_sb[dt][:, base + o2 : base + o2 + w2],
                                start=(dt == 0),
                                stop=(dt == 7),
                            )
                    nc.vector.tensor_scalar(
                        out=vt_sb[:, base : base + width],
                        in0=pt[:, :width],
                        scalar1=b_sb[:, 2:3],
                        scalar2=None,
                        op0=add,
                    )
                    nc.scalar.copy(
                        out=vt16_sb[:, base : base + width],
                        in_=vt_sb[:, base : base + width],
                    )
                    nc.sync.dma_start(
                        out=vt_out[:, base : base + width],
                        in_=vt16_sb[:, base : base + width],
                    )

                # V1[j] = [V natural | ones]  via PE transpose of VT slices
                for j in range(nkt):
                    vp = v1_psum.tile([P, QK], F16, tag="v1p")
                    nc.tensor.transpose(
                        vp, vt16_sb[:, j * P : (j + 1) * P], ident_sb[:QK, :QK]
                    )
                    nc.vector.tensor_copy(out=v1_sb[:, j, :QK], in_=vp)
                    nc.vector.memset(v1_sb[:, j, QK : QK + 1], 1.0)

            # ---- attention (flash-style over l_q halves) ----
            with (
                tc.tile_pool(name="st_psum", bufs=3, space="PSUM") as st_psum,
                tc.tile_pool(name="ot_psum", bufs=1, space="PSUM") as ot_psum,
                tc.tile_pool(name="et", bufs=nkt + 2) as et_pool,
            ):
                for half in range(2):
                    hb = half * (L // 2)
                    ets = []
                    # scores + exp for the whole half (independent of V/plm)
                    for j in range(nkt):
                        stp = st_psum.tile([P, L // 2], F32, tag="stp")
                        for o2, w2 in _chunks(L // 2, 512):
                            nc.tensor.matmul(
                                stp[:, o2 : o2 + w2],
                                lhsT=kt_sb[:, j * P : (j + 1) * P],
                                rhs=qt_sb[:, hb + o2 : hb + o2 + w2],
                                start=True,
                                stop=True,
                            )
                        et = et_pool.tile([P, L // 2], F16, tag="et")
                        nc.scalar.activation(
                            out=et,
                            in_=stp,
                            func=mybir.ActivationFunctionType.Exp,
                            bias=mask_sb[:, j : j + 1],
                            scale=NORM,
                        )
                        ets.append(et)
                    # O^T accumulation (needs V1, i.e. plm)
                    otp = ot_psum.tile([QK + 1, L // 2], F32, tag="otp")
                    for j in range(nkt):
                        for o2, w2 in _chunks(L // 2, 512):
                            nc.tensor.matmul(
                                otp[:, o2 : o2 + w2],
                                lhsT=v1_sb[:, j, :],
                                rhs=ets[j][:, o2 : o2 + w2],
                                start=(j == 0),
                                stop=(j == nkt - 1),
                            )
                    # 97-partition DMAs defeat the DMA-engine fanout (must be a
                    # multiple of 16): store rows 0..95 and the denom row apart.
                    for o2, w2 in _chunks(L // 2, 512):
                        nc.vector.tensor_copy(
                            out=ot_sb[:, hb + o2 : hb + o2 + w2],
                            in_=otp[:, o2 : o2 + w2],
                        )
                        nc.sync.dma_start(
                            out=ot_out[:QK, hb + o2 : hb + o2 + w2],
                            in_=ot_sb[:QK, hb + o2 : hb + o2 + w2],
                        )
                        nc.scalar.dma_start(
                            out=ot_out[QK : QK + 1, hb + o2 : hb + o2 + w2],
                            in_=ot_sb[QK : QK + 1, hb + o2 : hb + o2 + w2],
                        )
    nc.finalize()
    return nc


def _prep_core_inputs(evo, plm, seqlen, weights, nkt):
    evoT = np.zeros((Q_IN, L + 4), np.float16)
    evoT[:, 2 : 2 + L] = evo.T
    plmT = np.ascontiguousarray(plm.T.astype(np.float16))
    j = np.arange(nkt)[None, :]
    p = np.arange(P)[:, None]
    mask = np.where(j * P + p < seqlen, 0.0, -1e6).astype(np.float32)
    m = {"evoT": evoT, "plmT": plmT, "mask": mask}
    m.update(weights)
    return m


def _pack_w(w, n):
    # (n*128, 96) f32 -> (128, n*96) f16 in the SBUF [p, n, o] layout
    return np.ascontiguousarray(
        w.reshape(n, P, QK).transpose(1, 0, 2).reshape(P, n * QK).astype(np.float16)
    )


def kernel(
    plm_embedding,
    evo_local,
    seqlengths,
    q_w,
    q_b,
    k_w,
    k_b,
    v_w,
    v_b,
    cn3_w,
    cn3_b,
    cn5_w,
    cn5_b,
):
    global LAST_EXEC_TIME_NS, LAST_RESULTS
    plm_embedding = np.asarray(plm_embedding, np.float32)
    evo_local = np.asarray(evo_local, np.float32)
    seqlengths = np.asarray(seqlengths)

    taps, bk = _fold_k_weights(
        np.asarray(k_w, np.float32),
        np.asarray(k_b, np.float32),
        np.asarray(cn3_w, np.float32),
        np.asarray(cn3_b, np.float32),
        np.asarray(cn5_w, np.float32),
        np.asarray(cn5_b, np.float32),
    )
    nkt = int(min(L // P, (int(seqlengths.max()) + P - 1) // P))
    bqkv = np.stack(
        [np.asarray(q_b, np.float32), bk, np.asarray(v_b, np.float32)], axis=1
    ).astype(np.float32)
    weights = {
        "wq": _pack_w(np.ascontiguousarray(np.asarray(q_w, np.float32).T), 4),
        "wk": _pack_w(taps.reshape(5 * Q_IN, QK), 20),
        "wv": _pack_w(np.ascontiguousarray(np.asarray(v_w, np.float32).T), 8),
        "bqkv": np.ascontiguousarray(bqkv),
        "ident": np.eye(P, dtype=np.float16),
    }

    if nkt not in _program_cache:
        _program_cache[nkt] = _build_program(nkt)
    nc = _program_cache[nkt]

    in_maps = [
        _prep_core_inputs(evo_local[b], plm_embedding[b], int(seqlengths[b]), weights, nkt)
        for b in range(B)
    ]
    trace = bool(os.environ.get("KBENCH_TRACE"))
    res = run_bass_kernel_spmd(nc, in_maps, list(range(B)), trace=trace)
    LAST_EXEC_TIME_NS = res.exec_time_ns
    LAST_RESULTS = res

    out = np.empty((B, L, VD), np.float32)
    for b in range(B):
        ot = res.results[b]["ot"]
        vt = res.results[b]["vt"]
        out[b] = (ot[:QK] / ot[QK : QK + 1]).T + vt.T
    return out



# revision 10
# speedup vs baseline: 1.0961x; 1.0961x over previous
"""Contextual-attention Trainium2 kernel (Bass/Tile), data-parallel over batch.

Math (per sequence b):
    Q = evo @ q_w.T + q_b                                  (L, 96)
    K = cat(evo, conv3(evo), conv5(evo)) @ k_w.T + k_b     (L, 96)
    V = plm @ v_w.T + v_b                                  (L, 96)
    P = softmax(Q K^T / sqrt(96), key-masked by seqlen)
    out = P @ V + V

Device-side reformulation (per core = one sequence):
  * The two convs + concat + K-projection fold into 5 shifted matmuls:
        K[l] = sum_{t=-2..2} evo[l+t] @ taps[t]  + bk      (host-folded weights)
  * Q/K projections run in fp8e4 with perf_mode=DoubleRow (contraction 256 per
    matmul, 2x fewer PE instructions).  Weights are scaled x16 so fp8 stays in
    the normal range; the 1/256 descale folds into the softmax scale.
  * Transposed layout throughout ([feature, L]):
        QT = wq.T @ evoT, KT = taps.T @ evoT(shifted), VT = wv.T @ plmT (fp16)
        ST[lk, lq] = KT_slice.T @ QT per 512-query chunk
        exp: split between ScalarE (exact ACT Exp with -1e6 mask bias) and
        VectorE (Schraudolph: fp16 bit-pattern = round(x*1024/ln2 + 15329)
        written via an int16-viewed tensor_scalar, then a 0/1 per-partition
        mask multiply).  The constant exp bias cancels in the softmax ratio.
        OT[0:97] = sum_lk V1[lk].T @ ET[lk]  (V1 = [V | ones] via PE transpose)
  * Final divide, +V residual and the (96, L) -> (L, 96) transpose on host.
"""

import os
import numpy as np

import concourse.bacc as bacc
import concourse.bass as bass
import concourse.tile as tile
from concourse import mybir
from concourse._compat import get_trn_type
from concourse.bass_utils import run_bass_kernel_spmd

B, L = 8, 2048
Q_IN, V_IN, QK, VD = 512, 1024, 96, 96
P = 128
NORM = float(1.0 / np.sqrt(QK))
F32 = mybir.dt.float32
F16 = mybir.dt.float16
F8 = mybir.dt.float8e4
I16 = mybir.dt.int16
F8NP = mybir.dt.np(mybir.dt.float8e4)

WS = 16.0  # fp8 weight scale (lifts fp8 denormals); descale folded into NORM_EFF
NORM_EFF = NORM / (WS * WS)
A_EXP = float(NORM_EFF * 1024.0 / np.log(2.0))
B_EXP = 15360.0 - 31.0  # fp16 exponent bias * 1024, Schraudolph-centered
EVW = 2064  # padded evo columns (L + 4 -> multiple of 16)

LAST_EXEC_TIME_NS = None
LAST_RESULTS = None

_program_cache = {}


def _fold_k_weights(k_w, k_b, cn3_w, cn3_b, cn5_w, cn5_b):
    """K[l] = sum_{t in -2..2} evo[l+t] @ taps[t+2] + bk  (zero-padded shifts)."""
    A_evo = k_w[:, :Q_IN]
    A3 = k_w[:, Q_IN : Q_IN + VD]
    A5 = k_w[:, Q_IN + VD :]
    taps = np.zeros((5, Q_IN, QK), np.float32)
    for j in range(3):  # conv3 tap j acts at offset t = j-1
        taps[j - 1 + 2] += np.einsum("oc,cd->do", A3, cn3_w[:, :, j]).astype(np.float32)
    for j in range(5):  # conv5 tap j acts at offset t = j-2
        taps[j - 2 + 2] += np.einsum("oc,cd->do", A5, cn5_w[:, :, j]).astype(np.float32)
    taps[2] += A_evo.T
    bk = (k_b + A3 @ cn3_b + A5 @ cn5_b).astype(np.float32)
    return taps, bk


def _chunks(total, step=512):
    out = []
    o = 0
    while o < total:
        out.append((o, min(step, total - o)))
        o += step
    return out


def _scalar_js(nkt):
    # ScalarE gets the even tiles plus the last odd one (8 of 14); VectorE the rest
    s = set(range(0, nkt, 2))
    if nkt % 2 == 0 and nkt > 1:
        s.add(nkt - 1)
    return s


def _build_program(nkt):
    """One SPMD program; all cores run NKT key tiles, masks differ per core."""
    lkw = nkt * P
    DR = mybir.MatmulPerfMode.DoubleRow
    add = mybir.AluOpType.add
    mult = mybir.AluOpType.mult
    EXPF = mybir.ActivationFunctionType.Exp
    COPYF = mybir.ActivationFunctionType.Copy
    IDENTF = mybir.ActivationFunctionType.Identity
    scal_js = _scalar_js(nkt)

    nc = bacc.Bacc(get_trn_type() or "TRN2", target_bir_lowering=False, debug=False)
    # weights / constants
    wq8 = nc.declare_dram_parameter("wq8", [P, 2 * 2 * QK], F8, isOutput=False)
    wk8 = nc.declare_dram_parameter("wk8", [P, 5 * 2 * 2 * QK], F8, isOutput=False)
    wv = nc.declare_dram_parameter("wv", [P, 8 * QK], F16, isOutput=False)
    bqkv = nc.declare_dram_parameter("bqkv", [QK, 3], F32, isOutput=False)
    maskS = nc.declare_dram_parameter("maskS", [P, nkt], F32, isOutput=False)
    mask01 = nc.declare_dram_parameter("mask01", [P, nkt], F32, isOutput=False)
    identd = nc.declare_dram_parameter("ident", [P, P], F16, isOutput=False)
    # activations, chunk-major so each DMA is contiguous per partition
    evo8d = nc.declare_dram_parameter("evo8", [P, 4 * 2 * 2 * 528], F8, isOutput=False)
    plmd = nc.declare_dram_parameter("plm", [P, 4 * 8 * 512], F16, isOutput=False)
    # outputs
    ot_out = nc.declare_dram_parameter("ot", [QK + 1, L], F32, isOutput=True)
    vt_out = nc.declare_dram_parameter("vt", [QK, L], F16, isOutput=True)

    with tile.TileContext(nc) as tc:
        with tc.tile_pool(name="sing", bufs=1) as sing:
            # ---- input DMAs, in dependency-priority order ----
            wq_sb = sing.tile([P, 2, 2, QK], F8, tag="wq8")
            nc.sync.dma_start(
                out=wq_sb, in_=wq8[:, :].rearrange("p (a b o) -> p a b o", b=2, o=QK)
            )
            wk_sb = sing.tile([P, 5, 2, 2, QK], F8, tag="wk8")
            nc.sync.dma_start(
                out=wk_sb,
                in_=wk8[:, :].rearrange("p (t a b o) -> p t a b o", a=2, b=2, o=QK),
            )
            b_sb = sing.tile([QK, 3], F32, tag="bqkv")
            nc.sync.dma_start(out=b_sb, in_=bqkv[:, :])
            maskS_sb = sing.tile([P, nkt], F32, tag="maskS")
            nc.sync.dma_start(out=maskS_sb, in_=maskS[:, :])
            mask01_sb = sing.tile([P, nkt], F32, tag="mask01")
            nc.sync.dma_start(out=mask01_sb, in_=mask01[:, :])
            ident_sb = sing.tile([P, P], F16, tag="ident")
            nc.sync.dma_start(out=ident_sb, in_=identd[:, :])
            wv_sb = sing.tile([P, 8, QK], F16, tag="wv")
            nc.sync.dma_start(out=wv_sb, in_=wv[:, :].rearrange("p (n o) -> p n o", o=QK))

            # evo8 is stored chunk-major: [p][chunk][pair][j][528], chunk c
            # holding padded-evo columns [c*512, c*512+516) so every chunk is
            # self-contained for the 5 conv taps (+4 col overlap).
            evo_sb = sing.tile([P, 4, 2, 2, 528], F8, tag="evo8")
            evod = evo8d[:, :].rearrange("p (c a b w) -> p c a b w", a=2, b=2, w=528)
            for ci in range(4):
                nc.sync.dma_start(out=evo_sb[:, ci], in_=evod[:, ci])
            # plm chunk-major [p][chunk][cchunk][512]; on the scalar DGE queue
            # so input streams through both HW queues in parallel.
            plm_sb = sing.tile([P, 4, 8, 512], F16, tag="plm")
            plmdr = plmd[:, :].rearrange("p (c n w) -> p c n w", n=8, w=512)
            for ci in range(4):
                nc.scalar.dma_start(out=plm_sb[:, ci], in_=plmdr[:, ci])

            qt_sb = sing.tile([QK, L], F16, tag="qt")
            kt_sb = sing.tile([QK, lkw], F16, tag="kt")
            vt_sb = sing.tile([QK, L], F16, tag="vt")
            v1_sb = sing.tile([P, nkt, QK + 1], F16, tag="v1")
            scratch = sing.tile([QK, 1], F32, tag="scr")

            # preload the exp ACT table set during the DMA window
            nc.scalar.activation(out=scratch, in_=b_sb[:, 0:1], func=EXPF, scale=0.0)

            with (
                tc.tile_pool(name="proj_psum", bufs=3, space="PSUM") as proj_psum,
                tc.tile_pool(name="v1_psum", bufs=2, space="PSUM") as v1_psum,
            ):
                with nc.named_scope("proj_qk"):
                    # QT = wq.T @ evoT (+16*qb), fp8 DoubleRow
                    for ci in range(4):
                        base = ci * 512
                        pt = proj_psum.tile([QK, 512], F32, tag="proj")
                        for pair in range(2):
                            nc.tensor.matmul(
                                pt,
                                lhsT=wq_sb[:, pair],
                                rhs=evo_sb[:, ci, pair, :, 2:514],
                                start=(pair == 0),
                                stop=(pair == 1),
                                perf_mode=DR,
                            )
                        nc.vector.tensor_scalar(
                            out=qt_sb[:, base : base + 512],
                            in0=pt,
                            scalar1=b_sb[:, 0:1],
                            scalar2=None,
                            op0=add,
                        )
                    # KT = sum_t taps[t].T @ evoT(shift t-2) (+16*kb), fp8 DoubleRow
                    for base, width in _chunks(lkw, 512):
                        ci = base // 512
                        pt = proj_psum.tile([QK, 512], F32, tag="proj")
                        n = 0
                        for t in range(5):
                            for pair in range(2):
                                nc.tensor.matmul(
                                    pt[:, :width],
                                    lhsT=wk_sb[:, t, pair],
                                    rhs=evo_sb[:, ci, pair, :, t : t + width],
                                    start=(n == 0),
                                    stop=(n == 9),
                                    perf_mode=DR,
                                )
                                n += 1
                        nc.vector.tensor_scalar(
                            out=kt_sb[:, base : base + width],
                            in0=pt[:, :width],
                            scalar1=b_sb[:, 1:2],
                            scalar2=None,
                            op0=add,
                        )
                with nc.named_scope("proj_v"):
                    # VT = wv.T @ plmT (+vb), fp16; vb add + f16 cast on ScalarE
                    for ci in range(4):
                        base = ci * 512
                        pt = proj_psum.tile([QK, 512], F32, tag="proj")
                        for dt in range(8):
                            nc.tensor.matmul(
                                pt,
                                lhsT=wv_sb[:, dt, :],
                                rhs=plm_sb[:, ci, dt, :],
                                start=(dt == 0),
                                stop=(dt == 7),
                            )
                        nc.scalar.activation(
                            out=vt_sb[:, base : base + 512],
                            in_=pt,
                            func=IDENTF,
                            bias=b_sb[:, 2:3],
                            scale=1.0,
                        )
                        nc.scalar.dma_start(
                            out=vt_out[:, base : base + 512],
                            in_=vt_sb[:, base : base + 512],
                        )
                    # V1[j] = [V natural | ones] via PE transpose of VT slices
                    nc.vector.memset(v1_sb[:, :, QK : QK + 1], 1.0)
                    for j in range(nkt):
                        vp = v1_psum.tile([P, QK], F16, tag="v1p")
                        nc.tensor.transpose(
                            vp, vt_sb[:, j * P : (j + 1) * P], ident_sb[:QK, :QK]
                        )
                        nc.vector.tensor_copy(out=v1_sb[:, j, :QK], in_=vp)

            # ---- attention, flash-style per 512-query chunk ----
            with (
                tc.tile_pool(name="st_psum", bufs=5, space="PSUM") as st_psum,
                tc.tile_pool(name="ot_psum", bufs=2, space="PSUM") as ot_psum,
                tc.tile_pool(name="et", bufs=nkt + 2) as et_pool,
                tc.tile_pool(name="ot_sb", bufs=2) as ot_pool,
                nc.named_scope("attn"),
            ):
                for qc in range(4):
                    q0 = qc * 512
                    ets = []
                    for j in range(nkt):
                        stp = st_psum.tile([P, 512], F32, tag="stp")
                        nc.tensor.matmul(
                            stp,
                            lhsT=kt_sb[:, j * P : (j + 1) * P],
                            rhs=qt_sb[:, q0 : q0 + 512],
                            start=True,
                            stop=True,
                        )
                        et = et_pool.tile([P, 512], F16, tag="et")
                        if j in scal_js:
                            nc.scalar.activation(
                                out=et,
                                in_=stp,
                                func=EXPF,
                                bias=maskS_sb[:, j : j + 1],
                                scale=NORM_EFF,
                            )
                        else:
                            nc.vector.tensor_scalar(
                                out=et.bitcast(I16),
                                in0=stp,
                                scalar1=A_EXP,
                                scalar2=B_EXP,
                                op0=mult,
                                op1=add,
                            )
                            nc.vector.tensor_scalar(
                                out=et,
                                in0=et,
                                scalar1=mask01_sb[:, j : j + 1],
                                scalar2=None,
                                op0=mult,
                            )
                        ets.append(et)
                    otp = ot_psum.tile([QK + 1, 512], F32, tag="otp")
                    for j in range(nkt):
                        nc.tensor.matmul(
                            otp,
                            lhsT=v1_sb[:, j, :],
                            rhs=ets[j],
                            start=(j == 0),
                            stop=(j == nkt - 1),
                        )
                    ot_t = ot_pool.tile([QK + 1, 512], F32, tag="ot")
                    nc.scalar.activation(out=ot_t, in_=otp, func=COPYF, scale=1.0)
                    # 97-partition DMAs defeat the DMA-engine fanout (must be a
                    # multiple of 16): store rows 0..95 and the denom row apart.
                    nc.sync.dma_start(
                        out=ot_out[:QK, q0 : q0 + 512], in_=ot_t[:QK, :]
                    )
                    nc.sync.dma_start(
                        out=ot_out[QK : QK + 1, q0 : q0 + 512],
                        in_=ot_t[QK : QK + 1, :],
                    )
    nc.finalize()
    return nc


def _prep_core_inputs(evo, plm, seqlen, weights, nkt):
    ev = np.zeros((Q_IN, EVW), np.float32)
    ev[:, 2 : 2 + L] = evo.T
    # chunk-major [p][chunk][pair][j][528]; chunk c holds padded cols
    # [c*512, c*512+516) so conv taps never cross a chunk boundary
    ev4 = ev.reshape(2, 2, P, EVW)  # [pair][j][p][col]
    evo8 = np.zeros((P, 4, 2, 2, 528), F8NP)
    for c in range(4):
        cw = 516 if c < 3 else EVW - 1536
        evo8[:, c, :, :, :cw] = (
            ev4[:, :, :, c * 512 : c * 512 + cw].transpose(2, 0, 1, 3).astype(F8NP)
        )
    evo8 = np.ascontiguousarray(evo8.reshape(P, 4 * 2 * 2 * 528))
    # plm chunk-major [p][chunk][cchunk][512]
    plm16 = np.ascontiguousarray(
        plm.T.reshape(8, P, 4, 512).transpose(1, 2, 0, 3).reshape(P, 4 * 8 * 512)
    ).astype(np.float16)
    j = np.arange(nkt)[None, :]
    p = np.arange(P)[:, None]
    valid = j * P + p < seqlen
    maskS = np.where(valid, 0.0, -1e6).astype(np.float32)
    mask01 = valid.astype(np.float32)
    m = {"evo8": evo8, "plm": plm16, "maskS": maskS, "mask01": mask01}
    m.update(weights)
    return m


def _pack_w16(w, n):
    # (n*128, 96) f32 -> (128, n*96) f16 in the SBUF [p, n, o] layout
    return np.ascontiguousarray(
        w.reshape(n, P, QK).transpose(1, 0, 2).reshape(P, n * QK).astype(np.float16)
    )


def _pack_w8(w):
    # (512, 96) f32 -> (128, 2*2*96) fp8 in the SBUF [p, pair, j, o] layout
    return np.ascontiguousarray(
        (w * WS).reshape(2, 2, P, QK).transpose(2, 0, 1, 3).reshape(P, 4 * QK)
    ).astype(F8NP)


def kernel(
    plm_embedding,
    evo_local,
    seqlengths,
    q_w,
    q_b,
    k_w,
    k_b,
    v_w,
    v_b,
    cn3_w,
    cn3_b,
    cn5_w,
    cn5_b,
):
    global LAST_EXEC_TIME_NS, LAST_RESULTS
    plm_embedding = np.asarray(plm_embedding, np.float32)
    evo_local = np.asarray(evo_local, np.float32)
    seqlengths = np.asarray(seqlengths)

    taps, bk = _fold_k_weights(
        np.asarray(k_w, np.float32),
        np.asarray(k_b, np.float32),
        np.asarray(cn3_w, np.float32),
        np.asarray(cn3_b, np.float32),
        np.asarray(cn5_w, np.float32),
        np.asarray(cn5_b, np.float32),
    )
    nkt = int(min(L // P, (int(seqlengths.max()) + P - 1) // P))
    bqkv = np.stack(
        [
            WS * np.asarray(q_b, np.float32),
            WS * bk,
            np.asarray(v_b, np.float32),
        ],
        axis=1,
    ).astype(np.float32)
    wk8 = np.ascontiguousarray(
        (taps * WS).reshape(5, 2, 2, P, QK).transpose(3, 0, 1, 2, 4).reshape(P, 5 * 4 * QK)
    ).astype(F8NP)
    weights = {
        "wq8": _pack_w8(np.ascontiguousarray(np.asarray(q_w, np.float32).T)),
        "wk8": wk8,
        "wv": _pack_w16(np.ascontiguousarray(np.asarray(v_w, np.float32).T), 8),
        "bqkv": np.ascontiguousarray(bqkv),
        "ident": np.eye(P, dtype=np.float16),
    }

    if nkt not in _program_cache:
        _program_cache[nkt] = _build_program(nkt)
    nc = _program_cache[nkt]

    in_maps = [
        _prep_core_inputs(evo_local[b], plm_embedding[b], int(seqlengths[b]), weights, nkt)
        for b in range(B)
    ]
    trace = bool(os.environ.get("KBENCH_TRACE"))
    res = run_bass_kernel_spmd(nc, in_maps, list(range(B)), trace=trace)
    LAST_EXEC_TIME_NS = res.exec_time_ns
    LAST_RESULTS = res

    out = np.empty((B, L, VD), np.float32)
    for b in range(B):
        ot = res.results[b]["ot"]
        vt = res.results[b]["vt"]
        out[b] = (ot[:QK] / ot[QK : QK + 1]).T + vt.T
    return out


# revision 16
# speedup vs baseline: 1.1071x; 1.0101x over previous
"""Contextual-attention Trainium2 kernel (Bass/Tile), data-parallel over batch.

Math (per sequence b):
    Q = evo @ q_w.T + q_b                                  (L, 96)
    K = cat(evo, conv3(evo), conv5(evo)) @ k_w.T + k_b     (L, 96)
    V = plm @ v_w.T + v_b                                  (L, 96)
    P = softmax(Q K^T / sqrt(96), key-masked by seqlen)
    out = P @ V + V

Device-side reformulation (per core = one sequence):
  * The two convs + concat + K-projection fold into 5 shifted matmuls:
        K[l] = sum_{t=-2..2} evo[l+t] @ taps[t]  + bk      (host-folded weights)
  * Q/K projections run in fp8e4 with perf_mode=DoubleRow (contraction 256 per
    matmul, 2x fewer PE instructions).  Weights are scaled x16 so fp8 stays in
    the normal range; the 1/256 descale folds into the softmax scale.
  * Transposed layout throughout ([feature, L]):
        QT = wq.T @ evoT, KT = taps.T @ evoT(shifted), VT = wv.T @ plmT (fp16)
        ST[lk, lq] = KT_slice.T @ QT per 512-query chunk
        exp: split between ScalarE (exact ACT Exp with -1e6 mask bias) and
        VectorE (Schraudolph: fp16 bit-pattern = round(x*1024/ln2 + 15329)
        written via an int16-viewed tensor_scalar, then a 0/1 per-partition
        mask multiply).  The constant exp bias cancels in the softmax ratio.
        OT[0:97] = sum_lk V1[lk].T @ ET[lk]  (V1 = [V | ones] via PE transpose)
  * Final divide, +V residual and the (96, L) -> (L, 96) transpose on host.
"""

import os
import numpy as np

import concourse.bacc as bacc
import concourse.bass as bass
import concourse.tile as tile
from concourse import mybir
from concourse._compat import get_trn_type
from concourse.bass_utils import run_bass_kernel_spmd

B, L = 8, 2048
Q_IN, V_IN, QK, VD = 512, 1024, 96, 96
P = 128
NORM = float(1.0 / np.sqrt(QK))
F32 = mybir.dt.float32
F16 = mybir.dt.float16
F8 = mybir.dt.float8e4
I16 = mybir.dt.int16
F8NP = mybir.dt.np(mybir.dt.float8e4)

WS = 16.0  # fp8 weight scale (lifts fp8 denormals); descale folded into NORM_EFF
NORM_EFF = NORM / (WS * WS)
A_EXP = float(NORM_EFF * 1024.0 / np.log(2.0))
B_EXP = 15360.0 - 31.0  # fp16 exponent bias * 1024, Schraudolph-centered
EVW = 2064  # padded evo columns (L + 4 -> multiple of 16)

LAST_EXEC_TIME_NS = None
LAST_RESULTS = None

_program_cache = {}


def _fold_k_weights(k_w, k_b, cn3_w, cn3_b, cn5_w, cn5_b):
    """K[l] = sum_{t in -2..2} evo[l+t] @ taps[t+2] + bk  (zero-padded shifts)."""
    A_evo = k_w[:, :Q_IN]
    A3 = k_w[:, Q_IN : Q_IN + VD]
    A5 = k_w[:, Q_IN + VD :]
    taps = np.zeros((5, Q_IN, QK), np.float32)
    for j in range(3):  # conv3 tap j acts at offset t = j-1
        taps[j - 1 + 2] += np.einsum("oc,cd->do", A3, cn3_w[:, :, j]).astype(np.float32)
    for j in range(5):  # conv5 tap j acts at offset t = j-2
        taps[j - 2 + 2] += np.einsum("oc,cd->do", A5, cn5_w[:, :, j]).astype(np.float32)
    taps[2] += A_evo.T
    bk = (k_b + A3 @ cn3_b + A5 @ cn5_b).astype(np.float32)
    return taps, bk


def _chunks(total, step=512):
    out = []
    o = 0
    while o < total:
        out.append((o, min(step, total - o)))
        o += step
    return out


def _scalar_js(nkt):
    # ScalarE gets the even tiles plus the last odd one (8 of 14); VectorE the rest
    s = set(range(0, nkt, 2))
    if nkt % 2 == 0 and nkt > 1:
        s.add(nkt - 1)
    return s


def _build_program(nkt):
    """One SPMD program; all cores run NKT key tiles, masks differ per core."""
    lkw = nkt * P
    DR = mybir.MatmulPerfMode.DoubleRow
    add = mybir.AluOpType.add
    mult = mybir.AluOpType.mult
    EXPF = mybir.ActivationFunctionType.Exp
    COPYF = mybir.ActivationFunctionType.Copy
    IDENTF = mybir.ActivationFunctionType.Identity
    scal_js = _scalar_js(nkt)

    nc = bacc.Bacc(get_trn_type() or "TRN2", target_bir_lowering=False, debug=False)
    # all weights/constants in ONE fp8-typed blob (single fat DMA; tiny
    # separate transfers are descriptor-dominated): per-partition byte layout
    #   [0,384)      wq8   [2][2][96] fp8
    #   [384,2304)   wk8   [5][2][2][96] fp8
    #   [2304,3840)  wv    [8][96] f16
    #   [3840,4096)  ident [128] f16
    #   [4096,+4n)   maskS [nkt] f32
    #   [+4n,+8n)    mask01 [nkt] f32
    #   [+8n,+8n+12) bqkv  [3] f32 (rows 0..95 only)
    cbw = 4096 + 8 * nkt + 16
    blobd = nc.declare_dram_parameter("blob", [P, cbw], F8, isOutput=False)
    # activations, chunk-major so each DMA is contiguous per partition
    evo8d = nc.declare_dram_parameter("evo8", [P, 4 * 2 * 2 * 528], F8, isOutput=False)
    plmd = nc.declare_dram_parameter("plm", [P, 4 * 8 * 512], F16, isOutput=False)
    # outputs
    ot_out = nc.declare_dram_parameter("ot", [QK + 1, L], F32, isOutput=True)
    vt_out = nc.declare_dram_parameter("vt", [QK, L], F16, isOutput=True)

    with tile.TileContext(nc) as tc:
        with tc.tile_pool(name="sing", bufs=1) as sing:
            # ---- input DMAs, in dependency-priority order ----
            blob_sb = sing.tile([P, cbw], F8, tag="blob")
            nc.sync.dma_start(out=blob_sb, in_=blobd[:, :])
            wq_sb = blob_sb[:, 0:384].rearrange("p (a b o) -> p a b o", b=2, o=QK)
            wk_sb = blob_sb[:, 384:2304].rearrange(
                "p (t a b o) -> p t a b o", a=2, b=2, o=QK
            )
            wv_sb = blob_sb[:, 2304:3840].bitcast(F16).rearrange(
                "p (n o) -> p n o", o=QK
            )
            ident_sb = blob_sb[:, 3840:4096].bitcast(F16)
            maskS_sb = blob_sb[:, 4096 : 4096 + 4 * nkt].bitcast(F32)
            mask01_sb = blob_sb[:, 4096 + 4 * nkt : 4096 + 8 * nkt].bitcast(F32)
            b_sb = blob_sb[:QK, 4096 + 8 * nkt : 4096 + 8 * nkt + 12].bitcast(F32)

            # evo8 is stored chunk-major: [p][chunk][pair][j][528], chunk c
            # holding padded-evo columns [c*512, c*512+516) so every chunk is
            # self-contained for the 5 conv taps (+4 col overlap).
            evo_sb = sing.tile([P, 4, 2, 2, 528], F8, tag="evo8")
            evod = evo8d[:, :].rearrange("p (c a b w) -> p c a b w", a=2, b=2, w=528)
            for ci in range(4):
                nc.sync.dma_start(out=evo_sb[:, ci], in_=evod[:, ci])
            # plm chunk-major [p][chunk][cchunk][512]; on the scalar DGE queue
            # so input streams through both HW queues in parallel.
            plm_sb = sing.tile([P, 4, 8, 512], F16, tag="plm")
            plmdr = plmd[:, :].rearrange("p (c n w) -> p c n w", n=8, w=512)
            for ci in range(4):
                nc.scalar.dma_start(out=plm_sb[:, ci], in_=plmdr[:, ci])

            qt_sb = sing.tile([QK, L], F16, tag="qt")
            kt_sb = sing.tile([QK, lkw], F16, tag="kt")
            vt_sb = sing.tile([QK, L], F16, tag="vt")
            v1_sb = sing.tile([P, nkt, QK + 1], F16, tag="v1")
            scratch = sing.tile([QK, 1], F32, tag="scr")

            # preload the exp ACT table set during the DMA window
            nc.scalar.activation(out=scratch, in_=b_sb[:, 0:1], func=EXPF, scale=0.0)

            with (
                tc.tile_pool(name="proj_psum", bufs=3, space="PSUM") as proj_psum,
                tc.tile_pool(name="v1_psum", bufs=2, space="PSUM") as v1_psum,
            ):
                with nc.named_scope("proj_qk"):
                    # QT = wq.T @ evoT (+16*qb), fp8 DoubleRow
                    for ci in range(4):
                        base = ci * 512
                        pt = proj_psum.tile([QK, 512], F32, tag="proj")
                        for pair in range(2):
                            nc.tensor.matmul(
                                pt,
                                lhsT=wq_sb[:, pair],
                                rhs=evo_sb[:, ci, pair, :, 2:514],
                                start=(pair == 0),
                                stop=(pair == 1),
                                perf_mode=DR,
                            )
                        nc.vector.tensor_scalar(
                            out=qt_sb[:, base : base + 512],
                            in0=pt,
                            scalar1=b_sb[:, 0:1],
                            scalar2=None,
                            op0=add,
                        )
                    # KT = sum_t taps[t].T @ evoT(shift t-2) (+16*kb), fp8 DoubleRow
                    for base, width in _chunks(lkw, 512):
                        ci = base // 512
                        pt = proj_psum.tile([QK, 512], F32, tag="proj")
                        n = 0
                        for t in range(5):
                            for pair in range(2):
                                nc.tensor.matmul(
                                    pt[:, :width],
                                    lhsT=wk_sb[:, t, pair],
                                    rhs=evo_sb[:, ci, pair, :, t : t + width],
                                    start=(n == 0),
                                    stop=(n == 9),
                                    perf_mode=DR,
                                )
                                n += 1
                        nc.vector.tensor_scalar(
                            out=kt_sb[:, base : base + width],
                            in0=pt[:, :width],
                            scalar1=b_sb[:, 1:2],
                            scalar2=None,
                            op0=add,
                        )
                with nc.named_scope("proj_v"):
                    # VT = wv.T @ plmT (+vb), fp16; vb add + f16 cast on ScalarE
                    for ci in range(4):
                        base = ci * 512
                        pt = proj_psum.tile([QK, 512], F32, tag="proj")
                        for dt in range(8):
                            nc.tensor.matmul(
                                pt,
                                lhsT=wv_sb[:, dt, :],
                                rhs=plm_sb[:, ci, dt, :],
                                start=(dt == 0),
                                stop=(dt == 7),
                            )
                        nc.scalar.activation(
                            out=vt_sb[:, base : base + 512],
                            in_=pt,
                            func=IDENTF,
                            bias=b_sb[:, 2:3],
                            scale=1.0,
                        )
                        nc.scalar.dma_start(
                            out=vt_out[:, base : base + 512],
                            in_=vt_sb[:, base : base + 512],
                        )
                    # V1[j] = [V natural | ones] via PE transpose of VT slices
                    nc.vector.memset(v1_sb[:, :, QK : QK + 1], 1.0)
                    for j in range(nkt):
                        vp = v1_psum.tile([P, QK], F16, tag="v1p")
                        nc.tensor.transpose(
                            vp, vt_sb[:, j * P : (j + 1) * P], ident_sb[:QK, :QK]
                        )
                        nc.vector.tensor_copy(out=v1_sb[:, j, :QK], in_=vp)

            # ---- attention, flash-style per 512-query chunk; the O^T
            # accumulation is interleaved OT_LEAD tiles behind the scores so
            # the PE never waits on the exp engines ----
            OT_LEAD = min(4, max(1, nkt - 1))
            with (
                tc.tile_pool(name="st_psum", bufs=5, space="PSUM") as st_psum,
                tc.tile_pool(name="ot_psum", bufs=2, space="PSUM") as ot_psum,
                tc.tile_pool(name="et", bufs=OT_LEAD + 4) as et_pool,
                tc.tile_pool(name="ot_sb", bufs=2) as ot_pool,
                nc.named_scope("attn"),
            ):
                for qc in range(4):
                    q0 = qc * 512
                    ets = []
                    otp = ot_psum.tile([QK + 1, 512], F32, tag="otp")

                    def emit_ot(j):
                        nc.tensor.matmul(
                            otp,
                            lhsT=v1_sb[:, j, :],
                            rhs=ets[j],
                            start=(j == 0),
                            stop=(j == nkt - 1),
                            skip_group_check=True,
                        )

                    for j in range(nkt):
                        stp = st_psum.tile([P, 512], F32, tag="stp")
                        nc.tensor.matmul(
                            stp,
                            lhsT=kt_sb[:, j * P : (j + 1) * P],
                            rhs=qt_sb[:, q0 : q0 + 512],
                            start=True,
                            stop=True,
                        )
                        et = et_pool.tile([P, 512], F16, tag="et")
                        if j in scal_js:
                            nc.scalar.activation(
                                out=et,
                                in_=stp,
                                func=EXPF,
                                bias=maskS_sb[:, j : j + 1],
                                scale=NORM_EFF,
                            )
                        else:
                            nc.vector.tensor_scalar(
                                out=et.bitcast(I16),
                                in0=stp,
                                scalar1=A_EXP,
                                scalar2=B_EXP,
                                op0=mult,
                                op1=add,
                            )
                            nc.vector.tensor_scalar(
                                out=et,
                                in0=et,
                                scalar1=mask01_sb[:, j : j + 1],
                                scalar2=None,
                                op0=mult,
                            )
                        ets.append(et)
                        if j >= OT_LEAD:
                            emit_ot(j - OT_LEAD)
                    for j in range(nkt - OT_LEAD, nkt):
                        emit_ot(j)
                    ot_t = ot_pool.tile([QK + 1, 512], F32, tag="ot")
                    # alternate the PSUM->SBUF copy between the two free
                    # engines so neither becomes the attention bottleneck
                    if qc % 2 == 0:
                        nc.scalar.activation(out=ot_t, in_=otp, func=COPYF, scale=1.0)
                    else:
                        nc.vector.tensor_copy(out=ot_t, in_=otp)
                    # 97-partition DMAs defeat the DMA-engine fanout (must be a
                    # multiple of 16): store rows 0..95 and the denom row apart.
                    nc.sync.dma_start(
                        out=ot_out[:QK, q0 : q0 + 512], in_=ot_t[:QK, :]
                    )
                    nc.sync.dma_start(
                        out=ot_out[QK : QK + 1, q0 : q0 + 512],
                        in_=ot_t[QK : QK + 1, :],
                    )
    nc.finalize()
    return nc


def _build_program_bal(C, WX):
    """Balanced key-parallel SPMD program: every core serves TWO key-chunks
    (each = up to C key tiles of some sequence, host-assigned), computing
    partial softmax numerator+denominator over the full 2048 queries of the
    chunk's sequence.  The host sums partials per sequence.  V coverage for
    the +V residual beyond the chunk windows comes from a per-core `plmx`
    window that concatenates arbitrary (seq, col) tail pieces."""
    T = 2 * C
    CW = C * P
    KW = CW + 16
    NXC = WX // 512
    DR = mybir.MatmulPerfMode.DoubleRow
    add = mybir.AluOpType.add
    mult = mybir.AluOpType.mult
    EXPF = mybir.ActivationFunctionType.Exp
    COPYF = mybir.ActivationFunctionType.Copy
    IDENTF = mybir.ActivationFunctionType.Identity

    nc = bacc.Bacc(get_trn_type() or "TRN2", target_bir_lowering=False, debug=False)
    cbw = 4096 + 16 * C * 4 + 16
    blobd = nc.declare_dram_parameter("blob", [P, cbw], F8, isOutput=False)
    evoqa_d = nc.declare_dram_parameter("evoqa", [P, 4 * 2 * 2 * 528], F8, isOutput=False)
    evoqb_d = nc.declare_dram_parameter("evoqb", [P, 4 * 2 * 2 * 528], F8, isOutput=False)
    evoka_d = nc.declare_dram_parameter("evoka", [P, 2 * 2 * KW], F8, isOutput=False)
    evokb_d = nc.declare_dram_parameter("evokb", [P, 2 * 2 * KW], F8, isOutput=False)
    plma_d = nc.declare_dram_parameter("plma", [P, 8 * CW], F16, isOutput=False)
    plmb_d = nc.declare_dram_parameter("plmb", [P, 8 * CW], F16, isOutput=False)
    plmx_d = nc.declare_dram_parameter("plmx", [P, NXC * 8 * 512], F16, isOutput=False)
    ota_out = nc.declare_dram_parameter("ota", [QK + 1, L], F32, isOutput=True)
    otb_out = nc.declare_dram_parameter("otb", [QK + 1, L], F32, isOutput=True)
    vta_out = nc.declare_dram_parameter("vta", [QK, CW], F16, isOutput=True)
    vtb_out = nc.declare_dram_parameter("vtb", [QK, CW], F16, isOutput=True)
    vtx_out = nc.declare_dram_parameter("vtx", [QK, WX], F16, isOutput=True)

    with tile.TileContext(nc) as tc:
        with tc.tile_pool(name="sing", bufs=1) as sing:
            blob_sb = sing.tile([P, cbw], F8, tag="blob")
            nc.sync.dma_start(out=blob_sb, in_=blobd[:, :])
            wq_sb = blob_sb[:, 0:384].rearrange("p (a b o) -> p a b o", b=2, o=QK)
            wk_sb = blob_sb[:, 384:2304].rearrange(
                "p (t a b o) -> p t a b o", a=2, b=2, o=QK
            )
            wv_sb = blob_sb[:, 2304:3840].bitcast(F16).rearrange(
                "p (n o) -> p n o", o=QK
            )
            ident_sb = blob_sb[:, 3840:4096].bitcast(F16)
            maskS_sb = blob_sb[:, 4096 : 4096 + 4 * T].bitcast(F32)
            mask01_sb = blob_sb[:, 4096 + 4 * T : 4096 + 8 * T].bitcast(F32)
            boff = 4096 + 8 * T
            b_sb = blob_sb[:QK, boff : boff + 12].bitcast(F32)

            evoka_sb = sing.tile([P, 2, 2, KW], F8, tag="evoka")
            nc.sync.dma_start(
                out=evoka_sb,
                in_=evoka_d[:, :].rearrange("p (a b w) -> p a b w", a=2, w=KW),
            )
            evokb_sb = sing.tile([P, 2, 2, KW], F8, tag="evokb")
            nc.sync.dma_start(
                out=evokb_sb,
                in_=evokb_d[:, :].rearrange("p (a b w) -> p a b w", a=2, w=KW),
            )
            evoqa_sb = sing.tile([P, 4, 2, 2, 528], F8, tag="evoqa")
            evoqb_sb = sing.tile([P, 4, 2, 2, 528], F8, tag="evoqb")
            for d_, sb in ((evoqa_d, evoqa_sb), (evoqb_d, evoqb_sb)):
                dr = d_[:, :].rearrange("p (c a b w) -> p c a b w", a=2, b=2, w=528)
                for ci in range(4):
                    nc.sync.dma_start(out=sb[:, ci], in_=dr[:, ci])
            plma_sb = sing.tile([P, 8, CW], F16, tag="plma")
            nc.scalar.dma_start(
                out=plma_sb, in_=plma_d[:, :].rearrange("p (n w) -> p n w", w=CW)
            )
            plmb_sb = sing.tile([P, 8, CW], F16, tag="plmb")
            nc.scalar.dma_start(
                out=plmb_sb, in_=plmb_d[:, :].rearrange("p (n w) -> p n w", w=CW)
            )
            plmx_sb = sing.tile([P, NXC, 8, 512], F16, tag="plmx")
            plmx_dr = plmx_d[:, :].rearrange("p (c n w) -> p c n w", n=8, w=512)
            for ci in range(NXC):
                nc.scalar.dma_start(out=plmx_sb[:, ci], in_=plmx_dr[:, ci])

            qta_sb = sing.tile([QK, L], F16, tag="qta")
            qtb_sb = sing.tile([QK, L], F16, tag="qtb")
            kta_sb = sing.tile([QK, CW], F16, tag="kta")
            ktb_sb = sing.tile([QK, CW], F16, tag="ktb")
            vta_sb = sing.tile([QK, CW], F16, tag="vta")
            vtb_sb = sing.tile([QK, CW], F16, tag="vtb")
            vtx_sb = sing.tile([QK, WX], F16, tag="vtx")
            v1_sb = sing.tile([P, T, QK + 1], F16, tag="v1")
            scratch = sing.tile([QK, 1], F32, tag="scr")

            nc.scalar.activation(out=scratch, in_=b_sb[:, 0:1], func=EXPF, scale=0.0)

            with (
                tc.tile_pool(name="proj_psum", bufs=3, space="PSUM") as proj_psum,
                tc.tile_pool(name="v1_psum", bufs=2, space="PSUM") as v1_psum,
            ):
                with nc.named_scope("proj_qk"):
                    for evok, kt in ((evoka_sb, kta_sb), (evokb_sb, ktb_sb)):
                        for base, width in _chunks(CW, 512):
                            pt = proj_psum.tile([QK, 512], F32, tag="proj")
                            n = 0
                            for t in range(5):
                                for pair in range(2):
                                    nc.tensor.matmul(
                                        pt[:, :width],
                                        lhsT=wk_sb[:, t, pair],
                                        rhs=evok[:, pair, :, base + t : base + t + width],
                                        start=(n == 0),
                                        stop=(n == 9),
                                        perf_mode=DR,
                                    )
                                    n += 1
                            nc.vector.tensor_scalar(
                                out=kt[:, base : base + width],
                                in0=pt[:, :width],
                                scalar1=b_sb[:, 1:2],
                                scalar2=None,
                                op0=add,
                            )
                    for evoq, qt in ((evoqa_sb, qta_sb), (evoqb_sb, qtb_sb)):
                        for ci in range(4):
                            pt = proj_psum.tile([QK, 512], F32, tag="proj")
                            for pair in range(2):
                                nc.tensor.matmul(
                                    pt,
                                    lhsT=wq_sb[:, pair],
                                    rhs=evoq[:, ci, pair, :, 2:514],
                                    start=(pair == 0),
                                    stop=(pair == 1),
                                    perf_mode=DR,
                                )
                            nc.vector.tensor_scalar(
                                out=qt[:, ci * 512 : ci * 512 + 512],
                                in0=pt,
                                scalar1=b_sb[:, 0:1],
                                scalar2=None,
                                op0=add,
                            )
                with nc.named_scope("proj_v"):
                    vjobs = [
                        (plma_sb, vta_sb, vta_out, CW),
                        (plmb_sb, vtb_sb, vtb_out, CW),
                    ]
                    for plm_t, vt_t, vt_o, w_ in vjobs:
                        for base, width in _chunks(w_, 512):
                            pt = proj_psum.tile([QK, 512], F32, tag="proj")
                            for dt in range(8):
                                nc.tensor.matmul(
                                    pt[:, :width],
                                    lhsT=wv_sb[:, dt, :],
                                    rhs=plm_t[:, dt, base : base + width],
                                    start=(dt == 0),
                                    stop=(dt == 7),
                                )
                            nc.scalar.activation(
                                out=vt_t[:, base : base + width],
                                in_=pt[:, :width],
                                func=IDENTF,
                                bias=b_sb[:, 2:3],
                                scale=1.0,
                            )
                            nc.scalar.dma_start(
                                out=vt_o[:, base : base + width],
                                in_=vt_t[:, base : base + width],
                            )
                    for ci in range(NXC):
                        pt = proj_psum.tile([QK, 512], F32, tag="proj")
                        for dt in range(8):
                            nc.tensor.matmul(
                                pt,
                                lhsT=wv_sb[:, dt, :],
                                rhs=plmx_sb[:, ci, dt, :],
                                start=(dt == 0),
                                stop=(dt == 7),
                            )
                        nc.scalar.activation(
                            out=vtx_sb[:, ci * 512 : ci * 512 + 512],
                            in_=pt,
                            func=IDENTF,
                            bias=b_sb[:, 2:3],
                            scale=1.0,
                        )
                        nc.scalar.dma_start(
                            out=vtx_out[:, ci * 512 : ci * 512 + 512],
                            in_=vtx_sb[:, ci * 512 : ci * 512 + 512],
                        )
                    nc.vector.memset(v1_sb[:, :, QK : QK + 1], 1.0)
                    for j in range(T):
                        src = vta_sb if j < C else vtb_sb
                        col = (j % C) * P
                        vp = v1_psum.tile([P, QK], F16, tag="v1p")
                        nc.tensor.transpose(
                            vp, src[:, col : col + P], ident_sb[:QK, :QK]
                        )
                        nc.vector.tensor_copy(out=v1_sb[:, j, :QK], in_=vp)

            OT_LEAD = min(4, T - 1)
            with (
                tc.tile_pool(name="st_psum", bufs=4, space="PSUM") as st_psum,
                tc.tile_pool(name="ot_psum", bufs=4, space="PSUM") as ot_psum,
                tc.tile_pool(name="et", bufs=OT_LEAD + 4) as et_pool,
                tc.tile_pool(name="ot_sb", bufs=4) as ot_pool,
                nc.named_scope("attn"),
            ):
                for qc in range(4):
                    q0 = qc * 512
                    ets = []
                    otpA = ot_psum.tile([QK + 1, 512], F32, tag="otp")
                    otpB = ot_psum.tile([QK + 1, 512], F32, tag="otp")

                    def emit_ot(j):
                        nc.tensor.matmul(
                            otpA if j < C else otpB,
                            lhsT=v1_sb[:, j, :],
                            rhs=ets[j],
                            start=(j % C == 0),
                            stop=(j % C == C - 1),
                            skip_group_check=True,
                        )

                    for j in range(T):
                        kt = kta_sb if j < C else ktb_sb
                        qt = qta_sb if j < C else qtb_sb
                        col = (j % C) * P
                        stp = st_psum.tile([P, 512], F32, tag="stp")
                        nc.tensor.matmul(
                            stp,
                            lhsT=kt[:, col : col + P],
                            rhs=qt[:, q0 : q0 + 512],
                            start=True,
                            stop=True,
                        )
                        et = et_pool.tile([P, 512], F16, tag="et")
                        if j % 2 == 0:
                            nc.scalar.activation(
                                out=et,
                                in_=stp,
                                func=EXPF,
                                bias=maskS_sb[:, j : j + 1],
                                scale=NORM_EFF,
                            )
                        else:
                            nc.vector.tensor_scalar(
                                out=et.bitcast(I16),
                                in0=stp,
                                scalar1=A_EXP,
                                scalar2=B_EXP,
                                op0=mult,
                                op1=add,
                            )
                            nc.vector.tensor_scalar(
                                out=et,
                                in0=et,
                                scalar1=mask01_sb[:, j : j + 1],
                                scalar2=None,
                                op0=mult,
                            )
                        ets.append(et)
                        if j >= OT_LEAD:
                            emit_ot(j - OT_LEAD)
                    for j in range(T - OT_LEAD, T):
                        emit_ot(j)
                    for half, (otp, oo) in enumerate(
                        ((otpA, ota_out), (otpB, otb_out))
                    ):
                        ot_t = ot_pool.tile([QK + 1, 512], F32, tag="ot")
                        if (qc + half) % 2 == 0:
                            nc.scalar.activation(
                                out=ot_t, in_=otp, func=COPYF, scale=1.0
                            )
                        else:
                            nc.vector.tensor_copy(out=ot_t, in_=otp)
                        nc.sync.dma_start(
                            out=oo[:QK, q0 : q0 + 512], in_=ot_t[:QK, :]
                        )
                        nc.sync.dma_start(
                            out=oo[QK : QK + 1, q0 : q0 + 512],
                            in_=ot_t[QK : QK + 1, :],
                        )
    nc.finalize()
    return nc


def _prep_core_inputs(evo, plm, seqlen, weights, nkt):
    ev = np.zeros((Q_IN, EVW), np.float32)
    ev[:, 2 : 2 + L] = evo.T
    # chunk-major [p][chunk][pair][j][528]; chunk c holds padded cols
    # [c*512, c*512+516) so conv taps never cross a chunk boundary
    ev4 = ev.reshape(2, 2, P, EVW)  # [pair][j][p][col]
    evo8 = np.zeros((P, 4, 2, 2, 528), F8NP)
    for c in range(4):
        cw = 516 if c < 3 else EVW - 1536
        evo8[:, c, :, :, :cw] = (
            ev4[:, :, :, c * 512 : c * 512 + cw].transpose(2, 0, 1, 3).astype(F8NP)
        )
    evo8 = np.ascontiguousarray(evo8.reshape(P, 4 * 2 * 2 * 528))
    # plm chunk-major [p][chunk][cchunk][512]
    plm16 = np.ascontiguousarray(
        plm.T.reshape(8, P, 4, 512).transpose(1, 2, 0, 3).reshape(P, 4 * 8 * 512)
    ).astype(np.float16)
    j = np.arange(nkt)[None, :]
    p = np.arange(P)[:, None]
    valid = j * P + p < seqlen
    maskS = np.where(valid, 0.0, -1e6).astype(np.float32)
    mask01 = valid.astype(np.float32)
    blob = weights["blob_base"].copy()
    blob[:, 4096 : 4096 + 4 * nkt] = maskS.view(np.uint8).reshape(P, 4 * nkt)
    blob[:, 4096 + 4 * nkt : 4096 + 8 * nkt] = mask01.view(np.uint8).reshape(P, 4 * nkt)
    return {"evo8": evo8, "plm": plm16, "blob": blob.view(F8NP)}


def _pack_w16(w, n):
    # (n*128, 96) f32 -> (128, n*96) f16 in the SBUF [p, n, o] layout
    return np.ascontiguousarray(
        w.reshape(n, P, QK).transpose(1, 0, 2).reshape(P, n * QK).astype(np.float16)
    )


def _pack_w8(w):
    # (512, 96) f32 -> (128, 2*2*96) fp8 in the SBUF [p, pair, j, o] layout
    return np.ascontiguousarray(
        (w * WS).reshape(2, 2, P, QK).transpose(2, 0, 1, 3).reshape(P, 4 * QK)
    ).astype(F8NP)


def kernel(
    plm_embedding,
    evo_local,
    seqlengths,
    q_w,
    q_b,
    k_w,
    k_b,
    v_w,
    v_b,
    cn3_w,
    cn3_b,
    cn5_w,
    cn5_b,
):
    global LAST_EXEC_TIME_NS, LAST_RESULTS
    plm_embedding = np.asarray(plm_embedding, np.float32)
    evo_local = np.asarray(evo_local, np.float32)
    seqlengths = np.asarray(seqlengths)

    taps, bk = _fold_k_weights(
        np.asarray(k_w, np.float32),
        np.asarray(k_b, np.float32),
        np.asarray(cn3_w, np.float32),
        np.asarray(cn3_b, np.float32),
        np.asarray(cn5_w, np.float32),
        np.asarray(cn5_b, np.float32),
    )
    nkt = int(min(L // P, (int(seqlengths.max()) + P - 1) // P))
    bqkv = np.stack(
        [
            WS * np.asarray(q_b, np.float32),
            WS * bk,
            np.asarray(v_b, np.float32),
        ],
        axis=1,
    ).astype(np.float32)
    wk8 = np.ascontiguousarray(
        (taps * WS).reshape(5, 2, 2, P, QK).transpose(3, 0, 1, 2, 4).reshape(P, 5 * 4 * QK)
    ).astype(F8NP)
    wq8 = _pack_w8(np.ascontiguousarray(np.asarray(q_w, np.float32).T))
    wv16 = _pack_w16(np.ascontiguousarray(np.asarray(v_w, np.float32).T), 8)
    cbw = 4096 + 8 * nkt + 16
    blob = np.zeros((P, cbw), np.uint8)
    blob[:, 0:384] = wq8.view(np.uint8)
    blob[:, 384:2304] = wk8.view(np.uint8)
    blob[:, 2304:3840] = wv16.view(np.uint8).reshape(P, 1536)
    blob[:, 3840:4096] = np.eye(P, dtype=np.float16).view(np.uint8).reshape(P, 256)
    boff = 4096 + 8 * nkt
    blob[:QK, boff : boff + 12] = bqkv.view(np.uint8).reshape(QK, 12)
    weights = {"blob_base": blob}

    if nkt not in _program_cache:
        _program_cache[nkt] = _build_program(nkt)
    nc = _program_cache[nkt]

    in_maps = [
        _prep_core_inputs(evo_local[b], plm_embedding[b], int(seqlengths[b]), weights, nkt)
        for b in range(B)
    ]
    trace = bool(os.environ.get("KBENCH_TRACE"))
    res = run_bass_kernel_spmd(nc, in_maps, list(range(B)), trace=trace)
    LAST_EXEC_TIME_NS = res.exec_time_ns
    LAST_RESULTS = res

    out = np.empty((B, L, VD), np.float32)
    for b in range(B):
        ot = res.results[b]["ot"]
        vt = res.results[b]["vt"]
        out[b] = (ot[:QK] / ot[QK : QK + 1]).T + vt.T
    return out
